# revision 40
# baseline (speedup 1.0000x reference)
"""Trainium2 Bass kernel for nn_BaseSeg_Balance_Prior (segment_reduce).

Strategy: data-parallel over the flattened point axis P = B*N = 160000,
sharded 8 ways (20000 points/core, padded to 40 chunks x 512 points).

Per core, per 512-point chunk (activations feature-major [feat, pts]):
  L1 [7->256], L2 [256->512], L3 [512->256], L4 [256->128] as float32r
  matmuls (weights stationary, points moving, N=512 full PE rate),
  fused relu on ACT/DVE, PE transpose of the final [128feat, 512pt]
  tile to point-major, L2-normalize on DVE, per-class segment sums via
  onehot matmul into PSUM, and a [128pt, 4, 129] output tile (feat +
  label column) DMA'd straight into the right rows of out_cp.

The per-class partial sums [128feat, 16cls] are written per core; the
host reduces them across cores and finishes the tiny [13,128] EMA (the
"all-reduce + EMA" step: 8 x 6KB, host-side is faster than a device
collective's ~10us floor).
"""

import os
import sys

if "/opt/trn_rl_repo" not in sys.path:
    sys.path.insert(0, "/opt/trn_rl_repo")

import numpy as np

N_CORES = 8
P_TOTAL = 160000
PC = P_TOTAL // N_CORES      # 20000 points per core
CH = 512                     # points per chunk
NSUB = 4                     # 128-point subtiles per chunk
NC_CLS = 13
NCP = 16                     # padded class count
PAD_LABEL = 13               # pad points get a class outside 0..12

BETA = 0.999
MIN_PTS = 256
EPS = 1e-12

_CACHE = {}


def _build(n_chunks, pc, zero_bias, act_dt="bf16"):
    """Build + compile the per-core Bass program."""
    from contextlib import ExitStack

    import concourse.mybir as mybir
    import concourse.tile as tile
    from concourse import bacc
    from concourse.bass import ds, ts

    dt = mybir.dt
    f32 = dt.float32
    f32r = dt.float32r
    AL = mybir.AluOpType
    AF = mybir.ActivationFunctionType
    f16 = dt.float16
    adt = {"bf16": dt.bfloat16, "f16": f16}.get(act_dt, f32r)

    pp = n_chunks * CH           # padded per-core points
    tcols = pp // 128            # label columns in [128, tcols] layout

    nc = bacc.Bacc(
        "TRN2",
        target_bir_lowering=False,
        debug=False,
        enable_asserts=False,
        num_devices=N_CORES,
    )

    feats_d = nc.dram_tensor("feats", [7, pp], adt, kind="ExternalInput").ap()
    lblf_d = nc.dram_tensor("lblf", [128, tcols], f32, kind="ExternalInput").ap()
    w1_d = nc.dram_tensor("w1", [7, 256], adt, kind="ExternalInput").ap()
    w2_d = nc.dram_tensor("w2", [128, 2, 512], adt, kind="ExternalInput").ap()
    w3_d = nc.dram_tensor("w3", [128, 4, 256], adt, kind="ExternalInput").ap()
    w4_d = nc.dram_tensor("w4", [128, 2, 128], adt, kind="ExternalInput").ap()
    b1_d = nc.dram_tensor("b1s", [128, 2], f32, kind="ExternalInput").ap()
    b2_d = nc.dram_tensor("b2s", [128, 4], f32, kind="ExternalInput").ap()
    b3_d = nc.dram_tensor("b3s", [128, 2], f32, kind="ExternalInput").ap()
    b4_d = nc.dram_tensor("b4s", [128, 1], f32, kind="ExternalInput").ap()
    id_d = nc.dram_tensor("ident", [128, 128], f16, kind="ExternalInput").ap()
    oh_d = nc.dram_tensor("oh", [128, tcols, NCP], f16, kind="ExternalInput").ap()
    cp_d = nc.dram_tensor("out_cp", [pc, 129], f16, kind="ExternalOutput").ap()
    seg_d = nc.dram_tensor("out_seg", [NCP, 128], f32, kind="ExternalOutput").ap()

    with tile.TileContext(nc) as tc, ExitStack() as ctx:
        const = ctx.enter_context(tc.tile_pool(name="const", bufs=1))
        fpool = ctx.enter_context(tc.tile_pool(name="fin", bufs=6))
        apool = ctx.enter_context(tc.tile_pool(name="act", bufs=6))
        bpool = ctx.enter_context(tc.tile_pool(name="bout", bufs=6))
        spool = ctx.enter_context(tc.tile_pool(name="small", bufs=6))
        mmpool = ctx.enter_context(tc.tile_pool(name="mmp", bufs=3, space="PSUM"))
        tpool = ctx.enter_context(tc.tile_pool(name="tps", bufs=2, space="PSUM"))
        segpool = ctx.enter_context(tc.tile_pool(name="segp", bufs=1, space="PSUM"))

        # ---- constants ----
        w1_sb = const.tile([7, 256], adt)
        nc.sync.dma_start(w1_sb[:], w1_d[:])
        w2_sb = const.tile([128, 2, 512], adt)
        nc.sync.dma_start(w2_sb[:], w2_d[:])
        w3_sb = const.tile([128, 4, 256], adt)
        nc.sync.dma_start(w3_sb[:], w3_d[:])
        w4_sb = const.tile([128, 2, 128], adt)
        nc.sync.dma_start(w4_sb[:], w4_d[:])
        b1_sb = const.tile([128, 2], f32)
        nc.sync.dma_start(b1_sb[:], b1_d[:])
        b2_sb = const.tile([128, 4], f32)
        nc.sync.dma_start(b2_sb[:], b2_d[:])
        b3_sb = const.tile([128, 2], f32)
        nc.sync.dma_start(b3_sb[:], b3_d[:])
        b4_sb = const.tile([128, 1], f32)
        nc.sync.dma_start(b4_sb[:], b4_d[:])
        lblf_sb = const.tile([128, tcols], f32)
        ident = const.tile([128, 128], f16)
        nc.sync.dma_start(ident[:], id_d[:])
        oh_sb = const.tile([128, tcols, NCP], f16)
        seg_ps = segpool.tile([NCP, 128], f32, name="seg_ps")

        def relu3(engine, out3, in3, bias2, m_count):
            """out = relu(in + bias); in3/out3 are [128, m, CH]; per-m ops."""
            for m in range(m_count):
                if zero_bias:
                    if engine == "dve":
                        nc.vector.tensor_scalar(
                            out3[:, m, :], in3[:, m, :], 0.0, None, AL.max
                        )
                    else:
                        nc.scalar.activation(out3[:, m, :], in3[:, m, :], AF.Relu)
                else:
                    nc.scalar.activation(
                        out3[:, m, :], in3[:, m, :], AF.Relu, bias=bias2[:, m : m + 1]
                    )

        def relu_unit(engine, out2, in2, bias_ap):
            """out2 = relu(in2 + b); [128, CH] one PSUM bank -> SBUF."""
            if zero_bias:
                if engine == "dve":
                    nc.vector.tensor_scalar(out2, in2, 0.0, None, AL.max)
                else:
                    nc.scalar.activation(out2, in2, AF.Relu)
            else:
                nc.scalar.activation(out2, in2, AF.Relu, bias=bias_ap)

        def emit_seg(c, B, subs=range(NSUB)):
            """Per-class segment sums for chunk c (PSUM accumulation)."""
            for s in subs:
                nc.tensor.matmul(
                    seg_ps[:], oh_sb[:, c * NSUB + s, :], B[:, s, 0:128],
                    start=(c == 0 and s == 0),
                    stop=(c == n_chunks - 1 and s == NSUB - 1),
                    skip_group_check=True,
                )

        def stage_a(c, seg_fill=None):
            """DMA in, 4 MLP layers + relus, transpose."""
            f_ch = fpool.tile([7, CH], adt, tag="fch", name="f_ch")
            nc.sync.dma_start(f_ch[:], feats_d[:, ts(c, CH)])

            # L1: [7 -> 256] (2-bank psum tile, one merged relu)
            h = apool.tile([128, 2, CH], adt, tag="h", name="h")
            l1 = mmpool.tile([128, 2, CH], f32, tag="mm2", bufs=1, name="l1")
            for m in range(2):
                nc.tensor.matmul(
                    l1[:, m, :], w1_sb[:, ts(m, 128)], f_ch[:],
                    start=True, stop=True,
                )
            if zero_bias:
                nc.vector.tensor_scalar(h[:], l1[:], 0.0, None, AL.max)
            else:
                for m in range(2):
                    nc.scalar.activation(
                        h[:, m, :], l1[:, m, :], AF.Relu, bias=b1_sb[:, m : m + 1]
                    )

            # fill the reluH bubble on PE with lagged seg matmuls
            if seg_fill is not None:
                emit_seg(*seg_fill, subs=(0, 1))


            # L2: [256 -> 512]
            e = apool.tile([128, 4, CH], adt, tag="e", name="e")
            for m in range(4):
                l2 = mmpool.tile([128, CH], f32, tag="mm", name="l2")
                for kt in range(2):
                    nc.tensor.matmul(
                        l2[:], w2_sb[:, kt, ts(m, 128)], h[:, kt, :],
                        start=(kt == 0), stop=(kt == 1),
                    )
                relu_unit("act", e[:, m, :], l2[:], b2_sb[:, m : m + 1])

            # L3: [512 -> 256]
            g = apool.tile([128, 2, CH], adt, tag="g", name="g")
            for m in range(2):
                l3 = mmpool.tile([128, CH], f32, tag="mm", name="l3")
                for kt in range(4):
                    nc.tensor.matmul(
                        l3[:], w3_sb[:, kt, ts(m, 128)], e[:, kt, :],
                        start=(kt == 0), stop=(kt == 3),
                    )
                relu_unit("dve" if m == 0 else "act", g[:, m, :], l3[:],
                          b3_sb[:, m : m + 1])

            # fill the reluG bubble with the other half of the seg fill
            if seg_fill is not None:
                emit_seg(*seg_fill, subs=(2, 3))

            # L4: [256 -> 128] — shares the dedicated 2-bank slot with L1
            l4 = mmpool.tile([128, CH], f32, tag="mm2", bufs=1, name="l4")
            for kt in range(2):
                nc.tensor.matmul(
                    l4[:], w4_sb[:, kt, :], g[:, kt, :],
                    start=(kt == 0), stop=(kt == 1),
                )
            fF = apool.tile([128, CH], f16, tag="fF", name="fF")
            if zero_bias:
                nc.vector.tensor_scalar(fF[:, 0:256], l4[:, 0:256], 0.0, None, AL.max)
                nc.scalar.activation(fF[:, 256:512], l4[:, 256:512], AF.Relu)
            else:
                nc.scalar.activation(fF[:], l4[:], AF.Relu, bias=b4_sb[:, 0:1])

            # transpose to point-major [pt, feat]
            t_ps = tpool.tile([128, NSUB, 128], f16, name="t_ps")
            for s in range(NSUB):
                nc.tensor.transpose(
                    t_ps[:, s, :], fF[:, ts(s, 128)], ident[:]
                )
            return t_ps

        def b_pre(t_ps):
            """Norm chain for a finished chunk (inputs ready one iter ago)."""
            tf = apool.tile([128, NSUB, 128], f16, tag="tf", name="tf")
            nc.scalar.copy(tf[:], t_ps[:])
            sq = spool.tile([128, NSUB, 128], f32, tag="sq", name="sq")
            ssq = spool.tile([128, NSUB], f32, tag="ssq", name="ssq")
            nc.vector.tensor_tensor(sq[:], tf[:], tf[:], AL.mult)
            nc.vector.tensor_reduce(
                ssq[:], sq[:], axis=mybir.AxisListType.X, op=AL.add
            )
            nrm = spool.tile([128, NSUB], f32, tag="nrm", name="nrm")
            nc.scalar.sqrt(nrm[:], ssq[:])
            nrm2 = spool.tile([128, NSUB], f32, tag="nrm2", name="nrm2")
            nc.gpsimd.tensor_scalar(nrm2[:], nrm[:], EPS, None, AL.max)
            inv = spool.tile([128, NSUB], f32, tag="inv", name="inv")
            nc.vector.reciprocal(inv[:], nrm2[:])
            return tf, inv

        def b_post(c, tf, inv):
            """Normalized features + label column -> B, then DMA out."""
            B = bpool.tile([128, NSUB, 129], f16, name="B")
            for s in range(NSUB):
                if s == 3:
                    nc.scalar.mul(B[:, s, 0:128], tf[:, s, :], inv[:, s : s + 1])
                else:
                    nc.vector.tensor_scalar(
                        B[:, s, 0:128], tf[:, s, :], inv[:, s : s + 1], None, AL.mult
                    )
            nc.gpsimd.tensor_copy(B[:, :, 128], lblf_sb[:, ds(c * NSUB, NSUB)])

            full_rows = min(CH, pc - c * CH)
            if full_rows == CH:
                dst = cp_d[ts(c, CH), :].rearrange("(s p) f -> p s f", p=128)
                nc.sync.dma_start(dst, B[:, :, :])
            else:
                nsub_full = full_rows // 128
                rem = full_rows - nsub_full * 128
                if nsub_full:
                    dst = cp_d[ds(c * CH, nsub_full * 128), :].rearrange(
                        "(s p) f -> p s f", p=128
                    )
                    nc.sync.dma_start(dst, B[:, 0:nsub_full, :])
                if rem:
                    nc.sync.dma_start(
                        cp_d[ds(c * CH + nsub_full * 128, rem), :],
                        B[0:rem, nsub_full, :],
                    )
            return B

        pend = None
        Bs = {}
        for c in range(n_chunks):
            if c == 1:
                # big constants first needed by stage_b(0)/seg(0) — load
                # after chunk 0's input DMA so startup isn't gated on them
                nc.sync.dma_start(oh_sb[:], oh_d[:])
                nc.sync.dma_start(lblf_sb[:], lblf_d[:])
            seg_fill = (c - 2, Bs.pop(c - 2)) if c - 2 in Bs else None
            t_ps = stage_a(c, seg_fill)
            if pend is not None:
                Bs[pend[0]] = b_post(pend[0], *b_pre(pend[1]))
            pend = (c, t_ps)
        Bs[pend[0]] = b_post(pend[0], *b_pre(pend[1]))
        for c in sorted(Bs):
            emit_seg(c, Bs[c])

        seg_out = spool.tile([NCP, 128], f32, tag="segout", name="seg_out")
        nc.vector.tensor_copy(seg_out[:], seg_ps[:])
        nc.sync.dma_start(seg_d[:], seg_out[:])

    nc.compile()
    return nc


def _get_program(n_chunks, pc, zero_bias, act_dt="bf16"):
    key = (n_chunks, pc, zero_bias, act_dt)
    if key not in _CACHE:
        _CACHE[key] = _build(n_chunks, pc, zero_bias, act_dt)
    return _CACHE[key]


def _l2norm_np(v):
    n = np.sqrt(np.sum(v * v, axis=1, keepdims=True))
    return v / np.maximum(n, EPS)


def _conv_act(a, act_dt):
    if act_dt == "bf16":
        import ml_dtypes
        return np.ascontiguousarray(a, np.float32).astype(ml_dtypes.bfloat16)
    if act_dt == "f16":
        return np.ascontiguousarray(a, np.float32).astype(np.float16)
    return _tf32_round(a)


def _tf32_round(a):
    """Round fp32 to fp32r/TF32 (10-bit mantissa, round-to-nearest-even)."""
    a = np.ascontiguousarray(a, np.float32)
    u = a.view(np.uint32)
    bias = np.uint32(0xFFF) + ((u >> np.uint32(13)) & np.uint32(1))
    u2 = (u + bias) & np.uint32(0xFFFFE000)
    return u2.view(np.float32)


def kernel(pos, x, y, W_enc1, b_enc1, W_enc2, b_enc2, W1, b1, W2, b2, prior_ema):
    from concourse.bass_utils import run_bass_kernel_spmd

    f32 = np.float32
    pos = np.asarray(pos)
    x = np.asarray(x)
    y = np.asarray(y)
    B_, N_ = y.shape
    assert B_ * N_ == P_TOTAL

    n_chunks = (PC + CH - 1) // CH
    pp = n_chunks * CH
    tcols = pp // 128

    # ---- host prep: flatten + shard ----
    f0 = np.transpose(x, (0, 2, 1)).reshape(-1, x.shape[1]).astype(f32)
    p0 = pos.reshape(-1, pos.shape[-1]).astype(f32)
    all7 = np.ascontiguousarray(np.concatenate([f0, p0], axis=1).T)  # [7, P]
    labels = np.asarray(y).reshape(-1).astype(np.int64)
    counts = np.bincount(labels, minlength=NC_CLS).astype(np.float64)
    valid = counts >= MIN_PTS
    lbl_eff = np.where(valid[labels], labels, -1).astype(f32)

    act_dt = os.environ.get("KERNEL_DT", "f16")
    zero_bias = not (
        np.any(np.asarray(b_enc1)) or np.any(np.asarray(b_enc2))
        or np.any(np.asarray(b1)) or np.any(np.asarray(b2))
    )

    w1h = _conv_act(np.asarray(W_enc1, f32), act_dt)
    w2h = _conv_act(
        np.asarray(W_enc2, f32).reshape(2, 128, 512).transpose(1, 0, 2), act_dt
    )
    w3h = _conv_act(
        np.asarray(W1, f32).reshape(4, 128, 256).transpose(1, 0, 2), act_dt
    )
    w4h = _conv_act(
        np.asarray(W2, f32).reshape(2, 128, 128).transpose(1, 0, 2), act_dt
    )
    b1h = np.ascontiguousarray(np.asarray(b_enc1, f32).reshape(2, 128).T)
    b2h = np.ascontiguousarray(np.asarray(b_enc2, f32).reshape(4, 128).T)
    b3h = np.ascontiguousarray(np.asarray(b1, f32).reshape(2, 128).T)
    b4h = np.ascontiguousarray(np.asarray(b2, f32).reshape(1, 128).T)

    in_maps = []
    for c in range(N_CORES):
        sl = slice(c * PC, (c + 1) * PC)
        feats = np.zeros((7, pp), f32)
        feats[:, :PC] = all7[:, sl]
        feats = _conv_act(feats, act_dt)
        labc = np.full(pp, PAD_LABEL, np.int64)
        labc[:PC] = labels[sl]
        lab2d = labc.reshape(tcols, 128).T  # [128, tcols]
        onehot = (lab2d[:, :, None] == np.arange(NCP)[None, None, :]).astype(np.float16)
        lblc = np.zeros(pp, f32)
        lblc[:PC] = lbl_eff[sl]
        in_maps.append(
            {
                "feats": feats,
                "oh": np.ascontiguousarray(onehot),
                "lblf": np.ascontiguousarray(lblc.reshape(tcols, 128).T),
                "w1": w1h, "w2": w2h, "w3": w3h, "w4": w4h,
                "b1s": b1h, "b2s": b2h, "b3s": b3h, "b4s": b4h,
                "ident": np.eye(128, dtype=np.float16),
            }
        )

    nc = _get_program(n_chunks, PC, zero_bias, act_dt)
    trace = bool(int(os.environ.get("KERNEL_TRACE", "0")))
    if trace:
        _install_profhook()
    res = run_bass_kernel_spmd(
        nc, in_maps, core_ids=list(range(N_CORES)), trace=trace
    )
    kernel.last_exec_ns = res.exec_time_ns
    kernel.last_results = res

    # ---- host: gather + unshard ----
    cp = np.empty((P_TOTAL, 129), f32)
    seg_sum = np.zeros((NC_CLS, 128), np.float64)
    for c in range(N_CORES):
        rc = res.results[c]
        cp[c * PC : (c + 1) * PC] = rc["out_cp"].astype(f32)
        seg_sum += rc["out_seg"].astype(np.float64)[:NC_CLS, :]

    # EMA over the [13,128] prior (the tiny all-reduced tail)
    prior = np.asarray(prior_ema, np.float64)
    means = seg_sum / np.maximum(counts, 1.0)[:, None]
    cur = np.where(valid[:, None], means, prior)
    npr = BETA * prior + (1.0 - BETA) * cur
    nn_ = np.sqrt(np.sum(npr * npr, axis=1, keepdims=True))
    new_prior = (npr / np.maximum(nn_, EPS)).astype(f32)
    return cp, new_prior


kernel.last_exec_ns = None
kernel.last_results = None


def _install_profhook():
    """Make run_bass_kernel_spmd(trace=True) work in this container."""
    import types

    if "antenv.axon_hooks" not in sys.modules:
        mod = types.ModuleType("antenv.axon_hooks")
        mod._hook = None
        mod.set_axon_ntff_profile_hook = lambda h: setattr(mod, "_hook", h)
        mod.get_axon_ntff_profile_hook = lambda: mod._hook
        sys.modules["antenv.axon_hooks"] = mod
    try:
        from trn_agent_boot.trn_boot import _ntff_profile_via_ctypes

        hook = _ntff_profile_via_ctypes("/opt/axon/libaxon_pjrt.so")
        sys.modules["antenv.axon_hooks"].set_axon_ntff_profile_hook(hook)
    except Exception:
        pass
    import concourse.bass_utils as bu

    bu.upload_artifacts = lambda tmpdir: f"file://{tmpdir}"


# revision 42
# speedup vs baseline: 1.4888x; 1.4888x over previous
"""Trainium2 Bass kernel for nn_BaseSeg_Balance_Prior (segment_reduce).

Strategy: data-parallel over the flattened point axis P = B*N = 160000,
sharded 8 ways (20000 points/core, padded to 40 chunks x 512 points).

Per core, per 512-point chunk (activations feature-major [feat, pts]):
  L1 [7->256], L2 [256->512], L3 [512->256], L4 [256->128] as float32r
  matmuls (weights stationary, points moving, N=512 full PE rate),
  fused relu on ACT/DVE, PE transpose of the final [128feat, 512pt]
  tile to point-major, L2-normalize on DVE, per-class segment sums via
  onehot matmul into PSUM, and a [128pt, 4, 129] output tile (feat +
  label column) DMA'd straight into the right rows of out_cp.

The per-class partial sums [128feat, 16cls] are written per core; the
host reduces them across cores and finishes the tiny [13,128] EMA (the
"all-reduce + EMA" step: 8 x 6KB, host-side is faster than a device
collective's ~10us floor).
"""

import os
import sys

if "/opt/trn_rl_repo" not in sys.path:
    sys.path.insert(0, "/opt/trn_rl_repo")

import numpy as np

N_CORES = 8
P_TOTAL = 160000
PC = P_TOTAL // N_CORES      # 20000 points per core
CH = 512                     # points per chunk
NSUB = 4                     # 128-point subtiles per chunk
NC_CLS = 13
NCP = 16                     # padded class count
PAD_LABEL = 13               # pad points get a class outside 0..12

BETA = 0.999
MIN_PTS = 256
EPS = 1e-12

_CACHE = {}


def _build(n_chunks, pc, zero_bias, act_dt="bf16"):
    """Build + compile the per-core Bass program."""
    from contextlib import ExitStack

    import concourse.mybir as mybir
    import concourse.tile as tile
    from concourse import bacc
    from concourse.bass import ds, ts

    dt = mybir.dt
    f32 = dt.float32
    f32r = dt.float32r
    AL = mybir.AluOpType
    AF = mybir.ActivationFunctionType
    f16 = dt.float16
    adt = {"bf16": dt.bfloat16, "f16": f16}.get(act_dt, f32r)

    pp = n_chunks * CH           # padded per-core points
    tcols = pp // 128            # label columns in [128, tcols] layout

    nc = bacc.Bacc(
        "TRN2",
        target_bir_lowering=False,
        debug=False,
        enable_asserts=False,
        num_devices=N_CORES,
    )

    feats_d = nc.dram_tensor("feats", [7, pp], adt, kind="ExternalInput").ap()
    lblf_d = nc.dram_tensor("lblf", [128, tcols], f32, kind="ExternalInput").ap()
    w1_d = nc.dram_tensor("w1", [7, 256], adt, kind="ExternalInput").ap()
    w2_d = nc.dram_tensor("w2", [128, 2, 512], adt, kind="ExternalInput").ap()
    w3_d = nc.dram_tensor("w3", [128, 4, 256], adt, kind="ExternalInput").ap()
    w4_d = nc.dram_tensor("w4", [128, 2, 128], adt, kind="ExternalInput").ap()
    b1_d = nc.dram_tensor("b1s", [128, 2], f32, kind="ExternalInput").ap()
    b2_d = nc.dram_tensor("b2s", [128, 4], f32, kind="ExternalInput").ap()
    b3_d = nc.dram_tensor("b3s", [128, 2], f32, kind="ExternalInput").ap()
    b4_d = nc.dram_tensor("b4s", [128, 1], f32, kind="ExternalInput").ap()
    id_d = nc.dram_tensor("ident", [128, 128], f16, kind="ExternalInput").ap()
    oh_d = nc.dram_tensor("oh", [128, tcols, NCP], f16, kind="ExternalInput").ap()
    cp_d = nc.dram_tensor("out_cp", [pc, 129], f16, kind="ExternalOutput").ap()
    seg_d = nc.dram_tensor("out_seg", [NCP, 128], f32, kind="ExternalOutput").ap()

    with tile.TileContext(nc) as tc, ExitStack() as ctx:
        const = ctx.enter_context(tc.tile_pool(name="const", bufs=1))
        fpool = ctx.enter_context(tc.tile_pool(name="fin", bufs=6))
        apool = ctx.enter_context(tc.tile_pool(name="act", bufs=6))
        bpool = ctx.enter_context(tc.tile_pool(name="bout", bufs=6))
        spool = ctx.enter_context(tc.tile_pool(name="small", bufs=6))
        mmpool = ctx.enter_context(tc.tile_pool(name="mmp", bufs=3, space="PSUM"))
        tpool = ctx.enter_context(tc.tile_pool(name="tps", bufs=2, space="PSUM"))
        segpool = ctx.enter_context(tc.tile_pool(name="segp", bufs=1, space="PSUM"))

        # ---- constants ----
        w1_sb = const.tile([7, 256], adt)
        nc.sync.dma_start(w1_sb[:], w1_d[:])
        w2_sb = const.tile([128, 2, 512], adt)
        nc.sync.dma_start(w2_sb[:], w2_d[:])
        w3_sb = const.tile([128, 4, 256], adt)
        nc.sync.dma_start(w3_sb[:], w3_d[:])
        w4_sb = const.tile([128, 2, 128], adt)
        nc.sync.dma_start(w4_sb[:], w4_d[:])
        b1_sb = const.tile([128, 2], f32)
        nc.sync.dma_start(b1_sb[:], b1_d[:])
        b2_sb = const.tile([128, 4], f32)
        nc.sync.dma_start(b2_sb[:], b2_d[:])
        b3_sb = const.tile([128, 2], f32)
        nc.sync.dma_start(b3_sb[:], b3_d[:])
        b4_sb = const.tile([128, 1], f32)
        nc.sync.dma_start(b4_sb[:], b4_d[:])
        lblf_sb = const.tile([128, tcols], f32)
        ident = const.tile([128, 128], f16)
        nc.sync.dma_start(ident[:], id_d[:])
        oh_sb = const.tile([128, tcols, NCP], f16)
        seg_ps = segpool.tile([NCP, 128], f32, name="seg_ps")

        def relu3(engine, out3, in3, bias2, m_count):
            """out = relu(in + bias); in3/out3 are [128, m, CH]; per-m ops."""
            for m in range(m_count):
                if zero_bias:
                    if engine == "dve":
                        nc.vector.tensor_scalar(
                            out3[:, m, :], in3[:, m, :], 0.0, None, AL.max
                        )
                    else:
                        nc.scalar.activation(out3[:, m, :], in3[:, m, :], AF.Relu)
                else:
                    nc.scalar.activation(
                        out3[:, m, :], in3[:, m, :], AF.Relu, bias=bias2[:, m : m + 1]
                    )

        def relu_unit(engine, out2, in2, bias_ap):
            """out2 = relu(in2 + b); [128, CH] one PSUM bank -> SBUF."""
            if zero_bias:
                if engine == "dve":
                    nc.vector.tensor_scalar(out2, in2, 0.0, None, AL.max)
                else:
                    nc.scalar.activation(out2, in2, AF.Relu)
            else:
                nc.scalar.activation(out2, in2, AF.Relu, bias=bias_ap)

        def emit_seg(c, B, subs=range(NSUB)):
            """Per-class segment sums for chunk c (PSUM accumulation)."""
            for s in subs:
                nc.tensor.matmul(
                    seg_ps[:], oh_sb[:, c * NSUB + s, :], B[:, s, 0:128],
                    start=(c == 0 and s == 0),
                    stop=(c == n_chunks - 1 and s == NSUB - 1),
                    skip_group_check=True,
                )

        def stage_a(c, seg_fill=None):
            """DMA in, 4 MLP layers + relus, transpose."""
            f_ch = fpool.tile([7, CH], adt, tag="fch", name="f_ch")
            nc.sync.dma_start(f_ch[:], feats_d[:, ts(c, CH)])

            # L1: [7 -> 256] (2-bank psum tile, one merged relu)
            h = apool.tile([128, 2, CH], adt, tag="h", name="h")
            l1 = mmpool.tile([128, 2, CH], f32, tag="mm2", bufs=1, name="l1")
            for m in range(2):
                nc.tensor.matmul(
                    l1[:, m, :], w1_sb[:, ts(m, 128)], f_ch[:],
                    start=True, stop=True,
                )
            if zero_bias:
                nc.vector.tensor_scalar(h[:], l1[:], 0.0, None, AL.max)
            else:
                for m in range(2):
                    nc.scalar.activation(
                        h[:, m, :], l1[:, m, :], AF.Relu, bias=b1_sb[:, m : m + 1]
                    )

            # fill the reluH bubble on PE with lagged seg matmuls
            if seg_fill is not None:
                emit_seg(*seg_fill, subs=(0, 1))


            # L2: [256 -> 512]
            e = apool.tile([128, 4, CH], adt, tag="e", name="e")
            for m in range(4):
                l2 = mmpool.tile([128, CH], f32, tag="mm", name="l2")
                for kt in range(2):
                    nc.tensor.matmul(
                        l2[:], w2_sb[:, kt, ts(m, 128)], h[:, kt, :],
                        start=(kt == 0), stop=(kt == 1),
                    )
                relu_unit("act", e[:, m, :], l2[:], b2_sb[:, m : m + 1])

            # L3: [512 -> 256]
            g = apool.tile([128, 2, CH], adt, tag="g", name="g")
            for m in range(2):
                l3 = mmpool.tile([128, CH], f32, tag="mm", name="l3")
                for kt in range(4):
                    nc.tensor.matmul(
                        l3[:], w3_sb[:, kt, ts(m, 128)], e[:, kt, :],
                        start=(kt == 0), stop=(kt == 3),
                    )
                relu_unit("dve" if m == 0 else "act", g[:, m, :], l3[:],
                          b3_sb[:, m : m + 1])

            # fill the reluG bubble with the other half of the seg fill
            if seg_fill is not None:
                emit_seg(*seg_fill, subs=(2, 3))

            # L4: [256 -> 128]
            l4 = mmpool.tile([128, CH], f32, tag="mm", name="l4")
            for kt in range(2):
                nc.tensor.matmul(
                    l4[:], w4_sb[:, kt, :], g[:, kt, :],
                    start=(kt == 0), stop=(kt == 1),
                )
            fF = apool.tile([128, CH], f16, tag="fF", name="fF")
            if zero_bias:
                nc.vector.tensor_scalar(fF[:, 0:256], l4[:, 0:256], 0.0, None, AL.max)
                nc.scalar.activation(fF[:, 256:512], l4[:, 256:512], AF.Relu)
            else:
                nc.scalar.activation(fF[:], l4[:], AF.Relu, bias=b4_sb[:, 0:1])

            # transpose to point-major [pt, feat]
            t_ps = tpool.tile([128, NSUB, 128], f16, name="t_ps")
            for s in range(NSUB):
                nc.tensor.transpose(
                    t_ps[:, s, :], fF[:, ts(s, 128)], ident[:]
                )
            return t_ps

        def b_pre(t_ps):
            """Norm chain for a finished chunk (inputs ready one iter ago)."""
            tf = apool.tile([128, NSUB, 128], f16, tag="tf", name="tf")
            nc.scalar.copy(tf[:], t_ps[:])
            sq = spool.tile([128, NSUB, 128], f32, tag="sq", name="sq")
            ssq = spool.tile([128, NSUB], f32, tag="ssq", name="ssq")
            nc.vector.tensor_tensor(sq[:], tf[:], tf[:], AL.mult)
            nc.vector.tensor_reduce(
                ssq[:], sq[:], axis=mybir.AxisListType.X, op=AL.add
            )
            nrm = spool.tile([128, NSUB], f32, tag="nrm", name="nrm")
            nc.scalar.sqrt(nrm[:], ssq[:])
            nrm2 = spool.tile([128, NSUB], f32, tag="nrm2", name="nrm2")
            nc.gpsimd.tensor_scalar(nrm2[:], nrm[:], EPS, None, AL.max)
            inv = spool.tile([128, NSUB], f32, tag="inv", name="inv")
            nc.vector.reciprocal(inv[:], nrm2[:])
            return tf, inv

        def b_post(c, tf, inv):
            """Normalized features + label column -> B, then DMA out."""
            B = bpool.tile([128, NSUB, 129], f16, name="B")
            for s in range(NSUB):
                nc.vector.tensor_scalar(
                    B[:, s, 0:128], tf[:, s, :], inv[:, s : s + 1], None, AL.mult
                )
            nc.gpsimd.tensor_copy(B[:, :, 128], lblf_sb[:, ds(c * NSUB, NSUB)])

            full_rows = min(CH, pc - c * CH)
            if full_rows == CH:
                dst = cp_d[ts(c, CH), :].rearrange("(s p) f -> p s f", p=128)
                nc.sync.dma_start(dst, B[:, :, :])
            else:
                nsub_full = full_rows // 128
                rem = full_rows - nsub_full * 128
                if nsub_full:
                    dst = cp_d[ds(c * CH, nsub_full * 128), :].rearrange(
                        "(s p) f -> p s f", p=128
                    )
                    nc.sync.dma_start(dst, B[:, 0:nsub_full, :])
                if rem:
                    nc.sync.dma_start(
                        cp_d[ds(c * CH + nsub_full * 128, rem), :],
                        B[0:rem, nsub_full, :],
                    )
            return B

        pend = None
        Bs = {}
        for c in range(n_chunks):
            if c == 1:
                # big constants first needed by stage_b(0)/seg(0) — load
                # after chunk 0's input DMA so startup isn't gated on them
                nc.sync.dma_start(oh_sb[:], oh_d[:])
                nc.sync.dma_start(lblf_sb[:], lblf_d[:])
            seg_fill = (c - 2, Bs.pop(c - 2)) if c - 2 in Bs else None
            t_ps = stage_a(c, seg_fill)
            if pend is not None:
                Bs[pend[0]] = b_post(pend[0], *b_pre(pend[1]))
            pend = (c, t_ps)
        Bs[pend[0]] = b_post(pend[0], *b_pre(pend[1]))
        for c in sorted(Bs):
            emit_seg(c, Bs[c])

        seg_out = spool.tile([NCP, 128], f32, tag="segout", name="seg_out")
        nc.vector.tensor_copy(seg_out[:], seg_ps[:])
        nc.sync.dma_start(seg_d[:], seg_out[:])

    nc.compile()
    return nc


def _get_program(n_chunks, pc, zero_bias, act_dt="bf16"):
    key = (n_chunks, pc, zero_bias, act_dt)
    if key not in _CACHE:
        _CACHE[key] = _build(n_chunks, pc, zero_bias, act_dt)
    return _CACHE[key]


def _l2norm_np(v):
    n = np.sqrt(np.sum(v * v, axis=1, keepdims=True))
    return v / np.maximum(n, EPS)


def _conv_act(a, act_dt):
    if act_dt == "bf16":
        import ml_dtypes
        return np.ascontiguousarray(a, np.float32).astype(ml_dtypes.bfloat16)
    if act_dt == "f16":
        return np.ascontiguousarray(a, np.float32).astype(np.float16)
    return _tf32_round(a)


def _tf32_round(a):
    """Round fp32 to fp32r/TF32 (10-bit mantissa, round-to-nearest-even)."""
    a = np.ascontiguousarray(a, np.float32)
    u = a.view(np.uint32)
    bias = np.uint32(0xFFF) + ((u >> np.uint32(13)) & np.uint32(1))
    u2 = (u + bias) & np.uint32(0xFFFFE000)
    return u2.view(np.float32)


def kernel(pos, x, y, W_enc1, b_enc1, W_enc2, b_enc2, W1, b1, W2, b2, prior_ema):
    from concourse.bass_utils import run_bass_kernel_spmd

    f32 = np.float32
    pos = np.asarray(pos)
    x = np.asarray(x)
    y = np.asarray(y)
    B_, N_ = y.shape
    assert B_ * N_ == P_TOTAL

    n_chunks = (PC + CH - 1) // CH
    pp = n_chunks * CH
    tcols = pp // 128

    # ---- host prep: flatten + shard ----
    f0 = np.transpose(x, (0, 2, 1)).reshape(-1, x.shape[1]).astype(f32)
    p0 = pos.reshape(-1, pos.shape[-1]).astype(f32)
    all7 = np.ascontiguousarray(np.concatenate([f0, p0], axis=1).T)  # [7, P]
    labels = np.asarray(y).reshape(-1).astype(np.int64)
    counts = np.bincount(labels, minlength=NC_CLS).astype(np.float64)
    valid = counts >= MIN_PTS
    lbl_eff = np.where(valid[labels], labels, -1).astype(f32)

    act_dt = os.environ.get("KERNEL_DT", "f16")
    zero_bias = not (
        np.any(np.asarray(b_enc1)) or np.any(np.asarray(b_enc2))
        or np.any(np.asarray(b1)) or np.any(np.asarray(b2))
    )

    w1h = _conv_act(np.asarray(W_enc1, f32), act_dt)
    w2h = _conv_act(
        np.asarray(W_enc2, f32).reshape(2, 128, 512).transpose(1, 0, 2), act_dt
    )
    w3h = _conv_act(
        np.asarray(W1, f32).reshape(4, 128, 256).transpose(1, 0, 2), act_dt
    )
    w4h = _conv_act(
        np.asarray(W2, f32).reshape(2, 128, 128).transpose(1, 0, 2), act_dt
    )
    b1h = np.ascontiguousarray(np.asarray(b_enc1, f32).reshape(2, 128).T)
    b2h = np.ascontiguousarray(np.asarray(b_enc2, f32).reshape(4, 128).T)
    b3h = np.ascontiguousarray(np.asarray(b1, f32).reshape(2, 128).T)
    b4h = np.ascontiguousarray(np.asarray(b2, f32).reshape(1, 128).T)

    in_maps = []
    for c in range(N_CORES):
        sl = slice(c * PC, (c + 1) * PC)
        feats = np.zeros((7, pp), f32)
        feats[:, :PC] = all7[:, sl]
        feats = _conv_act(feats, act_dt)
        labc = np.full(pp, PAD_LABEL, np.int64)
        labc[:PC] = labels[sl]
        lab2d = labc.reshape(tcols, 128).T  # [128, tcols]
        onehot = (lab2d[:, :, None] == np.arange(NCP)[None, None, :]).astype(np.float16)
        lblc = np.zeros(pp, f32)
        lblc[:PC] = lbl_eff[sl]
        in_maps.append(
            {
                "feats": feats,
                "oh": np.ascontiguousarray(onehot),
                "lblf": np.ascontiguousarray(lblc.reshape(tcols, 128).T),
                "w1": w1h, "w2": w2h, "w3": w3h, "w4": w4h,
                "b1s": b1h, "b2s": b2h, "b3s": b3h, "b4s": b4h,
                "ident": np.eye(128, dtype=np.float16),
            }
        )

    nc = _get_program(n_chunks, PC, zero_bias, act_dt)
    trace = bool(int(os.environ.get("KERNEL_TRACE", "0")))
    if trace:
        _install_profhook()
    res = run_bass_kernel_spmd(
        nc, in_maps, core_ids=list(range(N_CORES)), trace=trace
    )
    kernel.last_exec_ns = res.exec_time_ns
    kernel.last_results = res

    # ---- host: gather + unshard ----
    cp = np.empty((P_TOTAL, 129), f32)
    seg_sum = np.zeros((NC_CLS, 128), np.float64)
    for c in range(N_CORES):
        rc = res.results[c]
        cp[c * PC : (c + 1) * PC] = rc["out_cp"].astype(f32)
        seg_sum += rc["out_seg"].astype(np.float64)[:NC_CLS, :]

    # EMA over the [13,128] prior (the tiny all-reduced tail)
    prior = np.asarray(prior_ema, np.float64)
    means = seg_sum / np.maximum(counts, 1.0)[:, None]
    cur = np.where(valid[:, None], means, prior)
    npr = BETA * prior + (1.0 - BETA) * cur
    nn_ = np.sqrt(np.sum(npr * npr, axis=1, keepdims=True))
    new_prior = (npr / np.maximum(nn_, EPS)).astype(f32)
    return cp, new_prior


kernel.last_exec_ns = None
kernel.last_results = None


def _install_profhook():
    """Make run_bass_kernel_spmd(trace=True) work in this container."""
    import types

    if "antenv.axon_hooks" not in sys.modules:
        mod = types.ModuleType("antenv.axon_hooks")
        mod._hook = None
        mod.set_axon_ntff_profile_hook = lambda h: setattr(mod, "_hook", h)
        mod.get_axon_ntff_profile_hook = lambda: mod._hook
        sys.modules["antenv.axon_hooks"] = mod
    try:
        from trn_agent_boot.trn_boot import _ntff_profile_via_ctypes

        hook = _ntff_profile_via_ctypes("/opt/axon/libaxon_pjrt.so")
        sys.modules["antenv.axon_hooks"].set_axon_ntff_profile_hook(hook)
    except Exception:
        pass
    import concourse.bass_utils as bu

    bu.upload_artifacts = lambda tmpdir: f"file://{tmpdir}"
